# revision 24
# baseline (speedup 1.0000x reference)
"""Causal self-attention on 8 Trainium2 NeuronCores.

Sharding: core = (batch b in {0,1}) x (head-group g in {0..3}), 4 heads per
core. Each core computes qkv for its heads from x[b], runs causal attention,
and multiplies by its 256 rows of w_proj, producing a partial output.
Host sums the 4 partials per batch (and transposes: the device writes out^T).

Layout: everything is computed "transposed" so no on-chip transposes are
needed. The host feeds x[b].T in bf16, pre-arranged partition-major so every
input lands in one line-rate DMA (strided 3D DMAs measured ~8x below line
rate); q^T/k^T come out of the qkv matmul with head-dim on partitions
(exactly the S^T = K Q^T operand layout); softmax runs on S^T (keys on
partitions, queries free) with the denominator obtained from a ones-column
appended to V in the A@V matmul; A@V's output Y^T feeds the projection with
w_proj as the stationary operand (one weight-load serves all queries); the
projection output is out^T, transposed back on the host.

Scheduling: one interleaved stream. Dummy warm-up matmuls run during the
input-DMA wait so the HAM clock gate is already 8/8 when real work starts; a
minimal prologue (first query-chunk of the q0/k0 bands + the first two v
tiles) starts the ACT-paced attention pipeline; all remaining qkv
band-chunks, v tiles, projections and normalizations are drip-fed into the
attention kt loop as PE filler under emission deadlines, so the tensor
engine never idles and no phase serializes against another. Normalization
is split per head-pair (pair 0's runs during pair 1's attention) and into a
DVE part (reciprocal) and a PE part (broadcast matmul) emitted a few
iterations apart, so the tail only carries pair 1's chain.
"""

import bisect
import numpy as np

import concourse.bass as bass
import concourse.bacc as bacc
import concourse.tile as tile
from concourse import mybir
from concourse.bass_utils import run_bass_kernel_spmd

F32 = mybir.dt.float32
BF16 = mybir.dt.bfloat16
EXP = mybir.ActivationFunctionType.Exp

B, T, C, H, HD = 2, 2048, 1024, 16, 64
NCORES = 8
HPC = 4      # heads per core
NPAIR = 2    # head pairs per core
NCT = C // 128   # 8 c-tiles
NTT = T // 128   # 16 t-tiles
NQC = T // 512   # 4 query chunks
SCALE = 1.0 / np.sqrt(HD)
NEG = -1.0e30
TCH = NCT * 512  # xT SBUF columns per query chunk (c-major within chunk)


def build_kernel():
    nc = bacc.Bacc("TRN2", target_bir_lowering=False, debug=False, num_devices=NCORES)

    # all large inputs are pre-arranged partition-major on the host: row p of
    # the dram tensor is exactly SBUF partition p's contents. wqkv is split
    # so the prologue-critical part (q0,k0 bands + v) loads first.
    xTp = nc.dram_tensor("xTp", [128, NQC * TCH], BF16, kind="ExternalInput")
    wqkv1p = nc.dram_tensor("wqkv1p", [128, NCT * 512], BF16, kind="ExternalInput")
    wqkv2p = nc.dram_tensor("wqkv2p", [128, NCT * 256], BF16, kind="ExternalInput")
    wpp = nc.dram_tensor("wpp", [128, 2 * C], BF16, kind="ExternalInput")
    maskc = nc.dram_tensor("maskc", [128, 256], F32, kind="ExternalInput")
    sel = nc.dram_tensor("sel", [2, 128], BF16, kind="ExternalInput")
    vones = nc.dram_tensor("vones", [128, 64], BF16, kind="ExternalInput")
    outT = nc.dram_tensor("outT", [C, T], BF16, kind="ExternalOutput")

    with tile.TileContext(nc) as tc:
        _body(tc, xTp, wqkv1p, wqkv2p, wpp, maskc, sel, vones, outT)

    nc.compile()
    return nc


def _body(tc, xTp, wqkv1p, wqkv2p, wpp, maskc, sel, vones, outT):
    nc = tc.nc
    from contextlib import ExitStack

    with ExitStack() as ctx:
        sb = lambda name, shape, dt: ctx.enter_context(
            tc.tile_pool(name=name, bufs=1)).tile(shape, dt, name=name)
        qkT_sb = sb("qkT", [128, 4 * T], BF16)        # bands q0,k0,q1,k1
        v65_sb = sb("v65", [128, NTT * 260], BF16)    # per k-tile: 4x(64 v + 1 ones)
        yt_sb = sb("yt", [128, NPAIR * T], BF16)      # pair p: rows 0-63 head 2p, 64-127 head 2p+1
        wp_sb = sb("wp", [128, 2 * C], BF16)
        maskc_sb = sb("maskc", [128, 256], F32)
        sel_sb = sb("sel", [2, 128], BF16)
        scr_sb = sb("scr", [128, 512], BF16)          # warm-up scratch
        ones_sb = sb("ones", [128, 8], BF16)          # denominator matmul lhsT
        # xT_sb column layout: qc-chunk-major, then c, then 512 t-columns
        xT_sb = sb("xT_sb", [128, NQC * TCH], BF16)
        wqkv1_sb = sb("wqkv1_sb", [128, NCT * 512], BF16)   # per c: q0|k0|v
        wqkv2_sb = sb("wqkv2_sb", [128, NCT * 256], BF16)   # per c: q1|k1

        es_pool = ctx.enter_context(tc.tile_pool(name="es", bufs=4))
        esum_pool = ctx.enter_context(tc.tile_pool(name="esum", bufs=2))
        esumb_pool = ctx.enter_context(tc.tile_pool(name="esumb", bufs=2))
        sums_pool = ctx.enter_context(tc.tile_pool(name="sums", bufs=2))
        rc_pool = ctx.enter_context(tc.tile_pool(name="rc", bufs=2))
        avst_pool = ctx.enter_context(tc.tile_pool(name="avst", bufs=2))
        ytr_pool = ctx.enter_context(tc.tile_pool(name="ytr", bufs=2))
        ost_pool = ctx.enter_context(tc.tile_pool(name="ost", bufs=6))

        # PSUM: psS 2x[128,1024] (4 banks) + av 2x[128,512] (2) + ps 2x[128,512] (2)
        ps = ctx.enter_context(tc.tile_pool(name="ps", bufs=2, space="PSUM"))
        av_pool = ctx.enter_context(tc.tile_pool(name="av", bufs=2, space="PSUM"))
        psS_pool = ctx.enter_context(tc.tile_pool(name="psS", bufs=2, space="PSUM"))

        v65_4d = v65_sb[:].rearrange("p (t h d) -> p t h d", t=NTT, h=HPC, d=65)

        def xA(c, t4):  # xT chunk for stage A: [128, 512] (c-tile, query chunk)
            return xT_sb[:, t4 * TCH + c * 512: t4 * TCH + (c + 1) * 512]

        def xB(c, t):   # xT tile for v: [128, 128] (c-tile, key tile)
            t4, r = divmod(t, 4)
            return xT_sb[:, t4 * TCH + c * 512 + r * 128: t4 * TCH + c * 512 + (r + 1) * 128]

        def w1(c, b):   # q0/k0 band chunk (b in 0,1) or v chunk (b == 2)
            w = 128 if b < 2 else 256
            return wqkv1_sb[:, c * 512 + b * 128: c * 512 + b * 128 + w]

        def w2(c, b):   # q1/k1 band chunk (b in 0,1)
            return wqkv2_sb[:, c * 256 + b * 128: c * 256 + (b + 1) * 128]

        # ---- input DMA stream (sync=HWDGE ring, FIFO). Everything
        # partition-major: long contiguous per-partition runs = line rate.
        nc.gpsimd.memset(scr_sb[:], 0.0)
        nc.gpsimd.memset(ones_sb[:], 1.0)
        nc.gpsimd.dma_start(
            v65_4d[:, :, :, 64:65],
            vones.ap().rearrange("p (t h o) -> p t h o", t=NTT, h=HPC, o=1))
        nc.sync.dma_start(wqkv1_sb[:], wqkv1p[:])
        nc.sync.dma_start(xT_sb[:, 0:TCH], xTp[:, 0:TCH])
        nc.sync.dma_start(maskc_sb[:], maskc[:])
        nc.sync.dma_start(sel_sb[:], sel[:])
        nc.sync.dma_start(wqkv2_sb[:], wqkv2p[:])
        for t4 in range(1, 4):
            nc.sync.dma_start(xT_sb[:, t4 * TCH:(t4 + 1) * TCH],
                              xTp[:, t4 * TCH:(t4 + 1) * TCH])
        nc.sync.dma_start(wp_sb[:], wpp[:])

        # warm-up: keep the PE busy during the input-DMA wait so the HAM
        # clock gate reaches 8/8 before (and stays through) the real work.
        for i in range(24):
            warm = psS_pool.tile([128, 1024], F32, tag="psS", name=f"warm_{i}")
            nc.tensor.matmul(warm[:, 0:512], scr_sb[:, 0:128], scr_sb[:],
                             start=True, stop=True)

        # ---- band task: one qk band (128 cols) x one query chunk (512) ----
        def emit_band(b, t4):
            acc = ps.tile([128, 512], F32, tag="ps", name=f"accA_{b}_{t4}")
            for c in range(NCT):
                lhs = w1(c, b) if b < 2 else w2(c, b - 2)
                nc.tensor.matmul(acc[:], lhs, xA(c, t4),
                                 start=(c == 0), stop=(c == NCT - 1))
            nc.vector.tensor_copy(qkT_sb[:, b * T + t4 * 512: b * T + (t4 + 1) * 512], acc[:])

        # ---- v task: v natural [t, j] for one k-tile (xT stationary) ----
        def emit_B(t):
            psv = ps.tile([128, 512], F32, tag="ps", name=f"psv_{t}")
            for c in range(NCT):
                nc.tensor.matmul(psv[:, 0:256], xB(c, t), w1(c, 2),
                                 start=(c == 0), stop=(c == NCT - 1))
            dst = v65_4d[:, t, :, 0:64]
            src_ = psv[:, 0:256].rearrange("p (h d) -> p h d", h=HPC, d=64)
            nc.vector.tensor_copy(dst, src_)

        # ---- projection group: out^T[n-chunk, q-chunk] with wp stationary --
        nproj = [0]
        tail_mode = [False]

        def emit_proj(qc, n):
            pso = ps.tile([128, 512], F32, tag="ps", name=f"pso_{qc}_{n}")
            for p in range(NPAIR):
                lhsT = wp_sb[:, p * C + n * 128: p * C + (n + 1) * 128]
                rhs = yt_sb[:, p * T + qc * 512: p * T + (qc + 1) * 512]
                nc.tensor.matmul(pso[:], lhsT, rhs, start=(p == 0), stop=(p == NPAIR - 1))
            ost = ost_pool.tile([128, 512], BF16, tag="ost", name=f"ost_{qc}_{n}")
            # in the tail the exp stream is done, so alternate the PSUM->SBUF
            # copy between DVE and the now-idle ACT engine so back-to-back
            # projection groups don't serialize on one copy engine.
            if tail_mode[0] and nproj[0] % 2 == 1:
                nc.scalar.copy(ost[:], pso[:])
            else:
                nc.vector.tensor_copy(ost[:], pso[:])
            eng = nc.gpsimd if (tail_mode[0] and nproj[0] % 2 == 0) else nc.sync
            nproj[0] += 1
            eng.dma_start(outT[n * 128:(n + 1) * 128, qc * 512:(qc + 1) * 512], ost[:])

        # ---- per-pair normalization, split into DVE part and PE part ----
        # everything lives on partitions 0:2 (the eviction DMA shifts the
        # sums rows there), so DVE ops stay partition-aligned and the sel
        # broadcast matrix is one shared [2,128] block.
        def norm_dve(qc, p, sums2, rcs):
            rc2f = rc_pool.tile([2, 512], F32, tag="rcf", name=f"rcf_{qc}_{p}")
            rc2b = rc_pool.tile([2, 512], BF16, tag="rcb", name=f"rcb_{qc}_{p}")
            nc.vector.reciprocal_approx_fast(rc2f[:], sums2[:])
            nc.vector.tensor_copy(rc2b[:], rc2f[:])
            rcs.append(rc2b)

        def norm_pe(qc, p, rcs, ytr):
            psR = ps.tile([128, 512], F32, tag="ps", name=f"psR_{p}_{qc}")
            nc.tensor.matmul(psR[:], sel_sb[0:2, 0:128], rcs[0][:],
                             start=True, stop=True)
            nc.vector.tensor_mul(yt_sb[:, p * T + qc * 512: p * T + (qc + 1) * 512],
                                 ytr[:], psR[:])

        # ---- filler scheduler: tasks sorted by emission deadline (seg, kt)
        # where seg = 2*qc+p of the attention segment and kt the iteration
        # within it before which the task MUST have been emitted (tile deps
        # only exist for already-emitted writers).
        tasks = []  # sorted list of (deadline, cost_ns, seq, fn)
        seq = [0]
        debt = [0.0]

        def add_task(dl, cost, fn):
            bisect.insort(tasks, (dl, cost, seq[0], fn))
            seq[0] += 1

        def run_head():
            dl, cost, _, fn = tasks.pop(0)
            fn()
            debt[0] -= cost

        def drip(credit):
            debt[0] += credit
            while tasks and debt[0] > 0:
                run_head()
            # one oversized task must not starve later drip slots
            debt[0] = max(debt[0], -1200.0)

        def force(dl):
            while tasks and tasks[0][0] <= dl:
                run_head()

        # prologue: q0/k0 bands for qc0 + first two v tiles
        emit_band(0, 0)
        emit_band(1, 0)
        emit_B(0)
        emit_B(1)
        add_task((0, 2), 1750, lambda: emit_B(2))
        add_task((0, 3), 1750, lambda: emit_B(3))
        for t4 in range(1, 4):
            for t in range(4 * t4, 4 * t4 + 4):
                add_task((2 * t4, t), 1750, lambda t=t: emit_B(t))
        for t4 in range(4):
            for p in range(NPAIR):
                if (t4, p) == (0, 0):
                    continue
                # q-band (2p) chunk t4 needed at segment (qc=t4, p) start
                add_task((2 * t4 + p, -1), 1900, lambda p=p, t4=t4: emit_band(2 * p, t4))
                # k-band (2p+1) chunk t4 needed when emit_S(4*t4) is emitted,
                # i.e. during iteration 4*t4-2 of segment (qc=t4, p)
                dlk = (p, -1) if t4 == 0 else (2 * t4 + p, 4 * t4 - 2)
                add_task(dlk, 1900, lambda p=p, t4=t4: emit_band(2 * p + 1, t4))

        # ---- attention: qc-outer, pair-inner; S^T -> exp -> A@V ----
        for qc in range(NQC):
            nkt = 4 * qc + 4
            for p in range(NPAIR):
                seg = 2 * qc + p
                force((seg, -1))
                qb, kb = 2 * p, 2 * p + 1
                av_t = av_pool.tile([128, 512], F32, tag="av", name=f"av_{p}_{qc}")
                esum = esum_pool.tile([128, 1024], F32, tag="esum", name=f"esum_{p}_{qc}")

                def emit_S(kt, qb=qb, kb=kb, qc=qc, p=p):
                    psb = psS_pool.tile([128, 1024], F32, tag="psS", name=f"psS_{p}_{qc}_{kt}")
                    slo = max(kt - 4 * qc, 0) * 128
                    for h in range(2):
                        base = 64 * h
                        lhsT = qkT_sb[base:base + 64, kb * T + kt * 128: kb * T + (kt + 1) * 128]
                        rhs = qkT_sb[base:base + 64, qb * T + qc * 512 + slo: qb * T + (qc + 1) * 512]
                        nc.tensor.matmul(psb[:, h * 512 + slo:(h + 1) * 512], lhsT, rhs,
                                         start=True, stop=True, tile_position=(base, 0))
                    return psb

                pipe = [emit_S(0)]
                if nkt > 1:
                    pipe.append(emit_S(1))
                for kt in range(nkt):
                    force((seg, kt))
                    cur = pipe.pop(0)
                    if kt + 2 < nkt:
                        pipe.append(emit_S(kt + 2))
                    d = kt - 4 * qc
                    lo = max(d, 0) * 128
                    psb2 = cur[:].rearrange("p (h q) -> p h q", h=2, q=512)
                    if d >= 0:
                        nc.vector.tensor_add(psb2[:, :, lo:lo + 128], psb2[:, :, lo:lo + 128],
                                             maskc_sb[:].rearrange("p (h q) -> p h q", h=2, q=128))
                    es = es_pool.tile([128, 1024], BF16, tag="es", name=f"es_{p}_{qc}_{kt}")
                    es2 = es[:].rearrange("p (h q) -> p h q", h=2, q=512)
                    nc.scalar.activation(es2[:, :, lo:], psb2[:, :, lo:], EXP, scale=SCALE)
                    for h in range(2):
                        hh = 2 * p + h
                        lhsT_v = v65_sb[:, kt * 260 + hh * 65: kt * 260 + hh * 65 + 64]
                        nc.tensor.matmul(av_t[64 * h:64 * (h + 1), lo:], lhsT_v,
                                         es[:, h * 512 + lo:(h + 1) * 512],
                                         start=(kt == 0), stop=(kt == nkt - 1))
                    if kt == 0:
                        nc.vector.tensor_copy(esum[:, 0:512], es[:, 0:512])
                        nc.gpsimd.tensor_copy(esum[:, 512:1024], es[:, 512:1024])
                    else:
                        nc.vector.tensor_add(esum[:, lo:512], esum[:, lo:512], es[:, lo:512])
                        nc.gpsimd.tensor_add(esum[:, 512 + lo:1024], esum[:, 512 + lo:1024],
                                             es[:, 512 + lo:1024])
                    drip(700.0 * (512 - lo) / 512)
                # evict Y^T + sums (PSUM can't feed DMA: stage via SBUF; the
                # ytr partition shift rides the sync ring, the sums rows ride
                # the gpsimd ring so the two don't serialize).
                ytr = ytr_pool.tile([128, 512], F32, tag="ytr", name=f"ytr_{p}_{qc}")
                sums2 = sums_pool.tile([2, 512], F32, tag="sums", name=f"sums_{qc}_{p}")
                nc.vector.tensor_copy(ytr[:], av_t[:])
                esumb = esumb_pool.tile([128, 1024], BF16, tag="esumb", name=f"esumb_{p}_{qc}")
                nc.scalar.copy(esumb[:], esum[:])
                psDs = []
                for h in range(2):
                    psD = ps.tile([128, 512], F32, tag="ps", name=f"psD_{p}_{qc}_{h}")
                    nc.tensor.matmul(psD[0:2, :], ones_sb[:, 0:2],
                                     esumb[:, h * 512:(h + 1) * 512], start=True, stop=True)
                    psDs.append(psD)
                # both rows of psD_h hold the denominator; single-partition
                # accesses must start at partition 0, so h0 copies straight
                # into sums2 row 0 and h1 stages at partition 0 + DMA-shifts
                nc.vector.tensor_copy(sums2[0:1, :], psDs[0][0:1, :])
                stD = avst_pool.tile([1, 512], F32, tag="stD", name=f"stD_{p}_{qc}")
                nc.vector.tensor_copy(stD[:], psDs[1][0:1, :])
                nc.sync.dma_start(sums2[1:2, :], stD[:])
                # pair p's normalization: DVE part one iteration into the
                # next segment (so the DVE queue head never blocks on the
                # sums-DMA latency), PE part a few iterations later. Pair 0's
                # whole chain runs during pair 1's attention; only pair 1's
                # lands after the last exp.
                rcs = []
                add_task((seg + 1, 1), 400,
                         lambda qc=qc, p=p, sums2=sums2, rcs=rcs: norm_dve(qc, p, sums2, rcs))
                add_task((seg + 1, 4), 900,
                         lambda qc=qc, p=p, rcs=rcs, ytr=ytr: norm_pe(qc, p, rcs, ytr))
            for n in range(NCT):
                add_task((2 * qc + 3, n), 550, lambda qc=qc, n=n: emit_proj(qc, n))
        for i in range(12):
            warm = psS_pool.tile([128, 1024], F32, tag="psS", name=f"tailwarm_{i}")
            nc.tensor.matmul(warm[:, 0:512], scr_sb[:, 0:128], scr_sb[:],
                             start=True, stop=True)
        tail_mode[0] = True
        force((1000, 0))


_NC_CACHE = None


def _get_nc():
    global _NC_CACHE
    if _NC_CACHE is None:
        _NC_CACHE = build_kernel()
    return _NC_CACHE


def _make_in_maps(x, w_attn, w_proj):
    import ml_dtypes
    bf16 = ml_dtypes.bfloat16
    x = np.asarray(x, dtype=np.float32)
    w_attn = np.asarray(w_attn, dtype=np.float32)
    w_proj = np.asarray(w_proj, dtype=np.float32)
    # maskc: strictly-lower-triangular NEG (row j = key, col i = query;
    # masked iff j > i), replicated for the 2 heads of a pair. Added before
    # the exp's scale is applied, so pre-divide by SCALE.
    tri = np.tril(np.full((128, 128), NEG, dtype=np.float32), -1) / SCALE
    maskc = np.concatenate([tri, tri], axis=1)
    sel = np.zeros((2, 128), dtype=np.float32)
    for m in range(128):
        sel[m // 64, m] = 1.0
    vones = np.ones((128, 64), dtype=bf16)
    sel = sel.astype(bf16)
    in_maps = []
    for core in range(NCORES):
        b, g = core // 4, core % 4
        hs = g * HPC
        q_cols = w_attn[:, hs * HD:(hs + HPC) * HD]
        k_cols = w_attn[:, C + hs * HD: C + (hs + HPC) * HD]
        v_cols = w_attn[:, 2 * C + hs * HD: 2 * C + (hs + HPC) * HD]
        # partition-major pre-arrangements (row p = SBUF partition p):
        # wqkv1[p, (c, q0|k0|v)] ; wqkv2[p, (c, q1|k1)] ; xTp[p, (t4, c, 512)]
        wqkv1 = np.concatenate(
            [q_cols[:, 0:128], k_cols[:, 0:128], v_cols], axis=1).astype(bf16)
        wqkv2 = np.concatenate(
            [q_cols[:, 128:256], k_cols[:, 128:256]], axis=1).astype(bf16)
        wqkv1p = wqkv1.reshape(NCT, 128, 512).transpose(1, 0, 2).reshape(128, NCT * 512)
        wqkv2p = wqkv2.reshape(NCT, 128, 256).transpose(1, 0, 2).reshape(128, NCT * 256)
        xT = np.ascontiguousarray(x[b].T).astype(bf16)          # [1024, 2048]
        xTp = xT.reshape(NCT, 128, NQC, 512).transpose(1, 2, 0, 3).reshape(128, NQC * NCT * 512)
        wpc = w_proj[hs * HD:(hs + HPC) * HD, :].astype(bf16)   # [256, 1024]
        wpp = wpc.reshape(2, 128, C).transpose(1, 0, 2).reshape(128, 2 * C)
        in_maps.append({
            "xTp": np.ascontiguousarray(xTp),
            "wqkv1p": np.ascontiguousarray(wqkv1p),
            "wqkv2p": np.ascontiguousarray(wqkv2p),
            "wpp": np.ascontiguousarray(wpp),
            "maskc": maskc,
            "sel": sel,
            "vones": vones,
        })
    return in_maps


def run_cores(x, w_attn, w_proj, trace=False):
    nc = _get_nc()
    in_maps = _make_in_maps(x, w_attn, w_proj)
    res = run_bass_kernel_spmd(nc, in_maps, core_ids=list(range(NCORES)), trace=trace)
    out = np.zeros((B, T, C), dtype=np.float32)
    for core in range(NCORES):
        out[core // 4] += np.asarray(res.results[core]["outT"], dtype=np.float32).T
    return out, res


def kernel(x, w_attn, w_proj):
    out, _ = run_cores(x, w_attn, w_proj, trace=False)
    return out


# revision 26
# speedup vs baseline: 1.3618x; 1.3618x over previous
"""Causal self-attention on 8 Trainium2 NeuronCores.

Sharding: core = (batch b in {0,1}) x (head-group g in {0..3}), 4 heads per
core. Each core computes qkv for its heads from x[b], runs causal attention,
and multiplies by its 256 rows of w_proj, producing a partial output.
Host sums the 4 partials per batch (and transposes: the device writes out^T).

Layout: everything is computed "transposed" so no on-chip transposes are
needed. The host feeds x[b].T in bf16, pre-arranged partition-major so every
input lands in one line-rate DMA (strided 3D DMAs measured ~8x below line
rate); q^T/k^T come out of the qkv matmul with head-dim on partitions
(exactly the S^T = K Q^T operand layout); softmax runs on S^T (keys on
partitions, queries free) with the denominator obtained from a ones-column
appended to V in the A@V matmul; A@V's output Y^T feeds the projection with
w_proj as the stationary operand (one weight-load serves all queries); the
projection output is out^T, transposed back on the host.

Scheduling: one interleaved stream. Dummy warm-up matmuls run during the
input-DMA wait so the HAM clock gate is already 8/8 when real work starts; a
minimal prologue (first query-chunk of the q0/k0 bands + the first two v
tiles) starts the ACT-paced attention pipeline; all remaining qkv
band-chunks, v tiles, projections and normalizations are drip-fed into the
attention kt loop as PE filler under emission deadlines, so the tensor
engine never idles and no phase serializes against another. Normalization
is split per head-pair (pair 0's runs during pair 1's attention) and into a
DVE part (reciprocal) and a PE part (broadcast matmul) emitted a few
iterations apart, so the tail only carries pair 1's chain.
"""

import bisect
import numpy as np

import concourse.bass as bass
import concourse.bacc as bacc
import concourse.tile as tile
from concourse import mybir
from concourse.bass_utils import run_bass_kernel_spmd

F32 = mybir.dt.float32
BF16 = mybir.dt.bfloat16
EXP = mybir.ActivationFunctionType.Exp

B, T, C, H, HD = 2, 2048, 1024, 16, 64
NCORES = 8
HPC = 4      # heads per core
NPAIR = 2    # head pairs per core
NCT = C // 128   # 8 c-tiles
NTT = T // 128   # 16 t-tiles
NQC = T // 512   # 4 query chunks
SCALE = 1.0 / np.sqrt(HD)
NEG = -1.0e30
TCH = NCT * 512  # xT SBUF columns per query chunk (c-major within chunk)


def build_kernel():
    nc = bacc.Bacc("TRN2", target_bir_lowering=False, debug=False, num_devices=NCORES)

    # all large inputs are pre-arranged partition-major on the host: row p of
    # the dram tensor is exactly SBUF partition p's contents. wqkv is split
    # so the prologue-critical part (q0,k0 bands + v) loads first.
    xTp = nc.dram_tensor("xTp", [128, NQC * TCH], BF16, kind="ExternalInput")
    wqkv1p = nc.dram_tensor("wqkv1p", [128, NCT * 512], BF16, kind="ExternalInput")
    wqkv2p = nc.dram_tensor("wqkv2p", [128, NCT * 256], BF16, kind="ExternalInput")
    wpp = nc.dram_tensor("wpp", [128, 2 * C], BF16, kind="ExternalInput")
    maskc = nc.dram_tensor("maskc", [128, 256], F32, kind="ExternalInput")
    sel = nc.dram_tensor("sel", [2, 128], BF16, kind="ExternalInput")
    vones = nc.dram_tensor("vones", [128, 64], BF16, kind="ExternalInput")
    outT = nc.dram_tensor("outT", [C, T], BF16, kind="ExternalOutput")

    with tile.TileContext(nc) as tc:
        _body(tc, xTp, wqkv1p, wqkv2p, wpp, maskc, sel, vones, outT)

    nc.compile()
    return nc


def _body(tc, xTp, wqkv1p, wqkv2p, wpp, maskc, sel, vones, outT):
    nc = tc.nc
    from contextlib import ExitStack

    with ExitStack() as ctx:
        sb = lambda name, shape, dt: ctx.enter_context(
            tc.tile_pool(name=name, bufs=1)).tile(shape, dt, name=name)
        qkT_sb = sb("qkT", [128, 4 * T], BF16)        # bands q0,k0,q1,k1
        v65_sb = sb("v65", [128, NTT * 260], BF16)    # per k-tile: 4x(64 v + 1 ones)
        yt_sb = sb("yt", [128, NPAIR * T], BF16)      # pair p: rows 0-63 head 2p, 64-127 head 2p+1
        wp_sb = sb("wp", [128, 2 * C], BF16)
        maskc_sb = sb("maskc", [128, 256], F32)
        sel_sb = sb("sel", [2, 128], BF16)
        scr_sb = sb("scr", [128, 512], BF16)          # warm-up scratch
        # xT_sb column layout: qc-chunk-major, then c, then 512 t-columns
        xT_sb = sb("xT_sb", [128, NQC * TCH], BF16)
        wqkv1_sb = sb("wqkv1_sb", [128, NCT * 512], BF16)   # per c: q0|k0|v
        wqkv2_sb = sb("wqkv2_sb", [128, NCT * 256], BF16)   # per c: q1|k1

        es_pool = ctx.enter_context(tc.tile_pool(name="es", bufs=4))
        sums_pool = ctx.enter_context(tc.tile_pool(name="sums", bufs=2))
        rc_pool = ctx.enter_context(tc.tile_pool(name="rc", bufs=2))
        avst_pool = ctx.enter_context(tc.tile_pool(name="avst", bufs=2))
        ytr_pool = ctx.enter_context(tc.tile_pool(name="ytr", bufs=2))
        ost_pool = ctx.enter_context(tc.tile_pool(name="ost", bufs=6))

        # PSUM: psS 2x[128,1024] (4 banks) + av 2x[128,512] (2) + ps 2x[128,512] (2)
        ps = ctx.enter_context(tc.tile_pool(name="ps", bufs=2, space="PSUM"))
        av_pool = ctx.enter_context(tc.tile_pool(name="av", bufs=2, space="PSUM"))
        psS_pool = ctx.enter_context(tc.tile_pool(name="psS", bufs=2, space="PSUM"))

        v65_4d = v65_sb[:].rearrange("p (t h d) -> p t h d", t=NTT, h=HPC, d=65)

        def xA(c, t4):  # xT chunk for stage A: [128, 512] (c-tile, query chunk)
            return xT_sb[:, t4 * TCH + c * 512: t4 * TCH + (c + 1) * 512]

        def xB(c, t):   # xT tile for v: [128, 128] (c-tile, key tile)
            t4, r = divmod(t, 4)
            return xT_sb[:, t4 * TCH + c * 512 + r * 128: t4 * TCH + c * 512 + (r + 1) * 128]

        def w1(c, b):   # q0/k0 band chunk (b in 0,1) or v chunk (b == 2)
            w = 128 if b < 2 else 256
            return wqkv1_sb[:, c * 512 + b * 128: c * 512 + b * 128 + w]

        def w2(c, b):   # q1/k1 band chunk (b in 0,1)
            return wqkv2_sb[:, c * 256 + b * 128: c * 256 + (b + 1) * 128]

        # ---- input DMA stream (sync=HWDGE ring, FIFO). Everything
        # partition-major: long contiguous per-partition runs = line rate.
        nc.gpsimd.memset(scr_sb[:], 0.0)
        nc.gpsimd.dma_start(
            v65_4d[:, :, :, 64:65],
            vones.ap().rearrange("p (t h o) -> p t h o", t=NTT, h=HPC, o=1))
        nc.sync.dma_start(wqkv1_sb[:, 0:2048], wqkv1p[:, 0:2048])
        nc.sync.dma_start(xT_sb[:, 0:2048], xTp[:, 0:2048])
        nc.sync.dma_start(wqkv1_sb[:, 2048:4096], wqkv1p[:, 2048:4096])
        nc.sync.dma_start(xT_sb[:, 2048:TCH], xTp[:, 2048:TCH])
        nc.sync.dma_start(maskc_sb[:], maskc[:])
        nc.sync.dma_start(sel_sb[:], sel[:])
        nc.sync.dma_start(wqkv2_sb[:], wqkv2p[:])
        for t4 in range(1, 4):
            nc.sync.dma_start(xT_sb[:, t4 * TCH:(t4 + 1) * TCH],
                              xTp[:, t4 * TCH:(t4 + 1) * TCH])
        nc.sync.dma_start(wp_sb[:], wpp[:])

        # warm-up: keep the PE busy during the input-DMA wait so the HAM
        # clock gate reaches 8/8 before (and stays through) the real work.
        for i in range(24):
            warm = psS_pool.tile([128, 1024], F32, tag="psS", name=f"warm_{i}")
            nc.tensor.matmul(warm[:, 0:512], scr_sb[:, 0:128], scr_sb[:],
                             start=True, stop=True)

        # ---- band task: one qk band (128 cols) x one query chunk (512) ----
        def emit_band(b, t4):
            acc = ps.tile([128, 512], F32, tag="ps", name=f"accA_{b}_{t4}")
            for c in range(NCT):
                lhs = w1(c, b) if b < 2 else w2(c, b - 2)
                nc.tensor.matmul(acc[:], lhs, xA(c, t4),
                                 start=(c == 0), stop=(c == NCT - 1))
            nc.vector.tensor_copy(qkT_sb[:, b * T + t4 * 512: b * T + (t4 + 1) * 512], acc[:])

        # ---- v task: v natural [t, j] for one k-tile (xT stationary) ----
        def emit_B(t):
            psv = ps.tile([128, 512], F32, tag="ps", name=f"psv_{t}")
            for c in range(NCT):
                nc.tensor.matmul(psv[:, 0:256], xB(c, t), w1(c, 2),
                                 start=(c == 0), stop=(c == NCT - 1))
            dst = v65_4d[:, t, :, 0:64]
            src_ = psv[:, 0:256].rearrange("p (h d) -> p h d", h=HPC, d=64)
            nc.vector.tensor_copy(dst, src_)

        # ---- projection group: out^T[n-chunk, q-chunk] with wp stationary --
        nproj = [0]
        tail_mode = [False]

        def emit_proj(qc, n):
            pso = ps.tile([128, 512], F32, tag="ps", name=f"pso_{qc}_{n}")
            for p in range(NPAIR):
                lhsT = wp_sb[:, p * C + n * 128: p * C + (n + 1) * 128]
                rhs = yt_sb[:, p * T + qc * 512: p * T + (qc + 1) * 512]
                nc.tensor.matmul(pso[:], lhsT, rhs, start=(p == 0), stop=(p == NPAIR - 1))
            ost = ost_pool.tile([128, 512], BF16, tag="ost", name=f"ost_{qc}_{n}")
            # in the tail the exp stream is done, so alternate the PSUM->SBUF
            # copy between DVE and the now-idle ACT engine so back-to-back
            # projection groups don't serialize on one copy engine.
            if tail_mode[0] and nproj[0] % 2 == 1:
                nc.scalar.copy(ost[:], pso[:])
            else:
                nc.vector.tensor_copy(ost[:], pso[:])
            eng = nc.sync if (tail_mode[0] and nproj[0] % 2 == 0) else nc.gpsimd
            nproj[0] += 1
            eng.dma_start(outT[n * 128:(n + 1) * 128, qc * 512:(qc + 1) * 512], ost[:])

        # ---- per-pair normalization, split into DVE part and PE part ----
        # everything lives on partitions 0:2 (the eviction DMA shifts the
        # sums rows there), so DVE ops stay partition-aligned and the sel
        # broadcast matrix is one shared [2,128] block.
        def norm_dve(qc, p, sums2, rcs):
            rc2f = rc_pool.tile([2, 512], F32, tag="rcf", name=f"rcf_{qc}_{p}")
            rc2b = rc_pool.tile([2, 512], BF16, tag="rcb", name=f"rcb_{qc}_{p}")
            nc.vector.reciprocal_approx_fast(rc2f[:], sums2[:])
            nc.vector.tensor_copy(rc2b[:], rc2f[:])
            rcs.append(rc2b)

        def norm_pe(qc, p, rcs, ytr):
            psR = ps.tile([128, 512], F32, tag="ps", name=f"psR_{p}_{qc}")
            nc.tensor.matmul(psR[:], sel_sb[0:2, 0:128], rcs[0][:],
                             start=True, stop=True)
            nc.vector.tensor_mul(yt_sb[:, p * T + qc * 512: p * T + (qc + 1) * 512],
                                 ytr[:], psR[:])

        # ---- filler scheduler: tasks sorted by emission deadline (seg, kt)
        # where seg = 2*qc+p of the attention segment and kt the iteration
        # within it before which the task MUST have been emitted (tile deps
        # only exist for already-emitted writers).
        tasks = []  # sorted list of (deadline, cost_ns, seq, fn)
        seq = [0]
        debt = [0.0]

        def add_task(dl, cost, fn):
            bisect.insort(tasks, (dl, cost, seq[0], fn))
            seq[0] += 1

        def run_head():
            dl, cost, _, fn = tasks.pop(0)
            fn()
            debt[0] -= cost

        def drip(credit):
            debt[0] += credit
            while tasks and debt[0] > 0:
                run_head()
            # one oversized task must not starve later drip slots
            debt[0] = max(debt[0], -1200.0)

        def force(dl):
            while tasks and tasks[0][0] <= dl:
                run_head()

        # prologue: q0/k0 bands for qc0 + first two v tiles
        emit_band(0, 0)
        emit_band(1, 0)
        emit_B(0)
        emit_B(1)
        add_task((0, 2), 1750, lambda: emit_B(2))
        add_task((0, 3), 1750, lambda: emit_B(3))
        for t4 in range(1, 4):
            for t in range(4 * t4, 4 * t4 + 4):
                add_task((2 * t4, t), 1750, lambda t=t: emit_B(t))
        for t4 in range(4):
            for p in range(NPAIR):
                if (t4, p) == (0, 0):
                    continue
                # q-band (2p) chunk t4 needed at segment (qc=t4, p) start
                add_task((2 * t4 + p, -1), 1900, lambda p=p, t4=t4: emit_band(2 * p, t4))
                # k-band (2p+1) chunk t4 needed when emit_S(4*t4) is emitted,
                # i.e. during iteration 4*t4-2 of segment (qc=t4, p)
                dlk = (p, -1) if t4 == 0 else (2 * t4 + p, 4 * t4 - 2)
                add_task(dlk, 1900, lambda p=p, t4=t4: emit_band(2 * p + 1, t4))

        # ---- attention: qc-outer, pair-inner; S^T -> exp -> A@V ----
        for qc in range(NQC):
            nkt = 4 * qc + 4
            for p in range(NPAIR):
                seg = 2 * qc + p
                force((seg, -1))
                qb, kb = 2 * p, 2 * p + 1
                av = [av_pool.tile([128, 512], F32, tag="av", name=f"av_{p}_{qc}_{i}")
                      for i in range(2)]

                def emit_S(kt, qb=qb, kb=kb, qc=qc, p=p):
                    psb = psS_pool.tile([128, 1024], F32, tag="psS", name=f"psS_{p}_{qc}_{kt}")
                    slo = max(kt - 4 * qc, 0) * 128
                    for h in range(2):
                        base = 64 * h
                        lhsT = qkT_sb[base:base + 64, kb * T + kt * 128: kb * T + (kt + 1) * 128]
                        rhs = qkT_sb[base:base + 64, qb * T + qc * 512 + slo: qb * T + (qc + 1) * 512]
                        nc.tensor.matmul(psb[:, h * 512 + slo:(h + 1) * 512], lhsT, rhs,
                                         start=True, stop=True, tile_position=(base, 0))
                    return psb

                pipe = [emit_S(0)]
                if nkt > 1:
                    pipe.append(emit_S(1))
                for kt in range(nkt):
                    force((seg, kt))
                    cur = pipe.pop(0)
                    if kt + 2 < nkt:
                        pipe.append(emit_S(kt + 2))
                    d = kt - 4 * qc
                    lo = max(d, 0) * 128
                    psb2 = cur[:].rearrange("p (h q) -> p h q", h=2, q=512)
                    if d >= 0:
                        nc.vector.tensor_add(psb2[:, :, lo:lo + 128], psb2[:, :, lo:lo + 128],
                                             maskc_sb[:].rearrange("p (h q) -> p h q", h=2, q=128))
                    es = es_pool.tile([128, 1024], BF16, tag="es", name=f"es_{p}_{qc}_{kt}")
                    es2 = es[:].rearrange("p (h q) -> p h q", h=2, q=512)
                    nc.scalar.activation(es2[:, :, lo:], psb2[:, :, lo:], EXP, scale=SCALE)
                    for h in range(2):
                        hh = 2 * p + h
                        lhsT_v = v65_sb[:, kt * 260 + hh * 65: kt * 260 + (hh + 1) * 65]
                        nc.tensor.matmul(av[h][0:65, lo:], lhsT_v, es[:, h * 512 + lo:(h + 1) * 512],
                                         start=(kt == 0), stop=(kt == nkt - 1))
                    drip(650.0 * (512 - lo) / 512)
                # evict Y^T + sums (PSUM can't feed DMA: stage via SBUF; the
                # ytr partition shift rides the sync ring, the sums rows ride
                # the gpsimd ring so the two don't serialize).
                for i in range(2):
                    bw = psS_pool.tile([128, 1024], F32, tag="psS", name=f"bwarm_{p}_{qc}_{i}")
                    nc.tensor.matmul(bw[:, 0:512], scr_sb[:, 0:128], scr_sb[:],
                                     start=True, stop=True)
                ytr = ytr_pool.tile([128, 512], F32, tag="ytr", name=f"ytr_{p}_{qc}")
                sums2 = sums_pool.tile([2, 512], F32, tag="sums", name=f"sums_{qc}_{p}")
                sts = []
                for h in range(2):
                    st = avst_pool.tile([65, 512], F32, tag="avst", name=f"avst_{p}_{qc}_{h}")
                    # h1's copy rides the ACT engine (idle at segment ends) so
                    # the two eviction copies run in parallel
                    if h == 0:
                        nc.vector.tensor_copy(st[:], av[h][0:65, :])
                    else:
                        nc.scalar.copy(st[:], av[h][0:65, :])
                    sts.append(st)
                # sums rows first on the sync ring: they gate the reciprocal
                # chain; the ytr shifts' latency is hidden by the schedule
                for h in range(2):
                    nc.sync.dma_start(sums2[h:h + 1, :], sts[h][64:65, :])
                for h in range(2):
                    nc.sync.dma_start(ytr[64 * h:64 * (h + 1), :], sts[h][0:64, :])
                # pair p's normalization: DVE part one iteration into the
                # next segment (so the DVE queue head never blocks on the
                # sums-DMA latency), PE part a few iterations later. Pair 0's
                # whole chain runs during pair 1's attention; only pair 1's
                # lands after the last exp.
                rcs = []
                add_task((seg + 1, 1), 400,
                         lambda qc=qc, p=p, sums2=sums2, rcs=rcs: norm_dve(qc, p, sums2, rcs))
                add_task((seg + 1, 4), 900,
                         lambda qc=qc, p=p, rcs=rcs, ytr=ytr: norm_pe(qc, p, rcs, ytr))
            for n in range(NCT):
                add_task((2 * qc + 3, n), 550, lambda qc=qc, n=n: emit_proj(qc, n))
        for i in range(12):
            warm = psS_pool.tile([128, 1024], F32, tag="psS", name=f"tailwarm_{i}")
            nc.tensor.matmul(warm[:, 0:512], scr_sb[:, 0:128], scr_sb[:],
                             start=True, stop=True)
        tail_mode[0] = True
        force((1000, 0))


_NC_CACHE = None


def _get_nc():
    global _NC_CACHE
    if _NC_CACHE is None:
        _NC_CACHE = build_kernel()
    return _NC_CACHE


def _make_in_maps(x, w_attn, w_proj):
    import ml_dtypes
    bf16 = ml_dtypes.bfloat16
    x = np.asarray(x, dtype=np.float32)
    w_attn = np.asarray(w_attn, dtype=np.float32)
    w_proj = np.asarray(w_proj, dtype=np.float32)
    # maskc: strictly-lower-triangular NEG (row j = key, col i = query;
    # masked iff j > i), replicated for the 2 heads of a pair. Added before
    # the exp's scale is applied, so pre-divide by SCALE.
    tri = np.tril(np.full((128, 128), NEG, dtype=np.float32), -1) / SCALE
    maskc = np.concatenate([tri, tri], axis=1)
    sel = np.zeros((2, 128), dtype=np.float32)
    for m in range(128):
        sel[m // 64, m] = 1.0
    vones = np.ones((128, 64), dtype=bf16)
    sel = sel.astype(bf16)
    in_maps = []
    for core in range(NCORES):
        b, g = core // 4, core % 4
        hs = g * HPC
        q_cols = w_attn[:, hs * HD:(hs + HPC) * HD]
        k_cols = w_attn[:, C + hs * HD: C + (hs + HPC) * HD]
        v_cols = w_attn[:, 2 * C + hs * HD: 2 * C + (hs + HPC) * HD]
        # partition-major pre-arrangements (row p = SBUF partition p):
        # wqkv1[p, (c, q0|k0|v)] ; wqkv2[p, (c, q1|k1)] ; xTp[p, (t4, c, 512)]
        wqkv1 = np.concatenate(
            [q_cols[:, 0:128], k_cols[:, 0:128], v_cols], axis=1).astype(bf16)
        wqkv2 = np.concatenate(
            [q_cols[:, 128:256], k_cols[:, 128:256]], axis=1).astype(bf16)
        wqkv1p = wqkv1.reshape(NCT, 128, 512).transpose(1, 0, 2).reshape(128, NCT * 512)
        wqkv2p = wqkv2.reshape(NCT, 128, 256).transpose(1, 0, 2).reshape(128, NCT * 256)
        xT = np.ascontiguousarray(x[b].T).astype(bf16)          # [1024, 2048]
        xTp = xT.reshape(NCT, 128, NQC, 512).transpose(1, 2, 0, 3).reshape(128, NQC * NCT * 512)
        wpc = w_proj[hs * HD:(hs + HPC) * HD, :].astype(bf16)   # [256, 1024]
        wpp = wpc.reshape(2, 128, C).transpose(1, 0, 2).reshape(128, 2 * C)
        in_maps.append({
            "xTp": np.ascontiguousarray(xTp),
            "wqkv1p": np.ascontiguousarray(wqkv1p),
            "wqkv2p": np.ascontiguousarray(wqkv2p),
            "wpp": np.ascontiguousarray(wpp),
            "maskc": maskc,
            "sel": sel,
            "vones": vones,
        })
    return in_maps


def run_cores(x, w_attn, w_proj, trace=False):
    nc = _get_nc()
    in_maps = _make_in_maps(x, w_attn, w_proj)
    res = run_bass_kernel_spmd(nc, in_maps, core_ids=list(range(NCORES)), trace=trace)
    out = np.zeros((B, T, C), dtype=np.float32)
    for core in range(NCORES):
        out[core // 4] += np.asarray(res.results[core]["outT"], dtype=np.float32).T
    return out, res


def kernel(x, w_attn, w_proj):
    out, _ = run_cores(x, w_attn, w_proj, trace=False)
    return out


# revision 27
# speedup vs baseline: 1.4020x; 1.0296x over previous
"""Causal self-attention on 8 Trainium2 NeuronCores.

Sharding: core = (batch b in {0,1}) x (head-group g in {0..3}), 4 heads per
core. Each core computes qkv for its heads from x[b], runs causal attention,
and multiplies by its 256 rows of w_proj, producing a partial output.
Host sums the 4 partials per batch (and transposes: the device writes out^T).

Layout: everything is computed "transposed" so no on-chip transposes are
needed. The host feeds x[b].T in bf16, pre-arranged partition-major so every
input lands in one line-rate DMA (strided 3D DMAs measured ~8x below line
rate); q^T/k^T come out of the qkv matmul with head-dim on partitions
(exactly the S^T = K Q^T operand layout); softmax runs on S^T (keys on
partitions, queries free) with the denominator obtained from a ones-column
appended to V in the A@V matmul; A@V's output Y^T feeds the projection with
w_proj as the stationary operand (one weight-load serves all queries); the
projection output is out^T, transposed back on the host.

Scheduling: one interleaved stream. Dummy warm-up matmuls run during the
input-DMA wait so the HAM clock gate is already 8/8 when real work starts; a
minimal prologue (first query-chunk of the q0/k0 bands + the first two v
tiles) starts the ACT-paced attention pipeline; all remaining qkv
band-chunks, v tiles, projections and normalizations are drip-fed into the
attention kt loop as PE filler under emission deadlines, so the tensor
engine never idles and no phase serializes against another. Normalization
is split per head-pair (pair 0's runs during pair 1's attention) and into a
DVE part (reciprocal) and a PE part (broadcast matmul) emitted a few
iterations apart, so the tail only carries pair 1's chain.
"""

import bisect
import numpy as np

import concourse.bass as bass
import concourse.bacc as bacc
import concourse.tile as tile
from concourse import mybir
from concourse.bass_utils import run_bass_kernel_spmd

F32 = mybir.dt.float32
BF16 = mybir.dt.bfloat16
EXP = mybir.ActivationFunctionType.Exp

B, T, C, H, HD = 2, 2048, 1024, 16, 64
NCORES = 8
HPC = 4      # heads per core
NPAIR = 2    # head pairs per core
NCT = C // 128   # 8 c-tiles
NTT = T // 128   # 16 t-tiles
NQC = T // 512   # 4 query chunks
SCALE = 1.0 / np.sqrt(HD)
NEG = -1.0e30
TCH = NCT * 512  # xT SBUF columns per query chunk (c-major within chunk)


def build_kernel():
    nc = bacc.Bacc("TRN2", target_bir_lowering=False, debug=False, num_devices=NCORES)

    # all large inputs are pre-arranged partition-major on the host: row p of
    # the dram tensor is exactly SBUF partition p's contents. wqkv is split
    # so the prologue-critical part (q0,k0 bands + v) loads first.
    xTp = nc.dram_tensor("xTp", [128, NQC * TCH], BF16, kind="ExternalInput")
    wqkv1p = nc.dram_tensor("wqkv1p", [128, NCT * 512], BF16, kind="ExternalInput")
    wqkv2p = nc.dram_tensor("wqkv2p", [128, NCT * 256], BF16, kind="ExternalInput")
    wpp = nc.dram_tensor("wpp", [128, 2 * C], BF16, kind="ExternalInput")
    maskc = nc.dram_tensor("maskc", [128, 256], F32, kind="ExternalInput")
    sel = nc.dram_tensor("sel", [2, 128], BF16, kind="ExternalInput")
    vones = nc.dram_tensor("vones", [128, 64], BF16, kind="ExternalInput")
    outT = nc.dram_tensor("outT", [C, T], BF16, kind="ExternalOutput")

    with tile.TileContext(nc) as tc:
        _body(tc, xTp, wqkv1p, wqkv2p, wpp, maskc, sel, vones, outT)

    nc.compile()
    return nc


def _body(tc, xTp, wqkv1p, wqkv2p, wpp, maskc, sel, vones, outT):
    nc = tc.nc
    from contextlib import ExitStack

    with ExitStack() as ctx:
        sb = lambda name, shape, dt: ctx.enter_context(
            tc.tile_pool(name=name, bufs=1)).tile(shape, dt, name=name)
        qkT_sb = sb("qkT", [128, 4 * T], BF16)        # bands q0,k0,q1,k1
        v65_sb = sb("v65", [128, NTT * 260], BF16)    # per k-tile: 4x(64 v + 1 ones)
        yt_sb = sb("yt", [128, NPAIR * T], BF16)      # pair p: rows 0-63 head 2p, 64-127 head 2p+1
        wp_sb = sb("wp", [128, 2 * C], BF16)
        maskc_sb = sb("maskc", [128, 256], F32)
        sel_sb = sb("sel", [2, 128], BF16)
        scr_sb = sb("scr", [128, 512], BF16)          # warm-up scratch
        # xT_sb column layout: qc-chunk-major, then c, then 512 t-columns
        xT_sb = sb("xT_sb", [128, NQC * TCH], BF16)
        wqkv1_sb = sb("wqkv1_sb", [128, NCT * 512], BF16)   # per c: q0|k0|v
        wqkv2_sb = sb("wqkv2_sb", [128, NCT * 256], BF16)   # per c: q1|k1

        es_pool = ctx.enter_context(tc.tile_pool(name="es", bufs=4))
        sums_pool = ctx.enter_context(tc.tile_pool(name="sums", bufs=2))
        rc_pool = ctx.enter_context(tc.tile_pool(name="rc", bufs=2))
        avst_pool = ctx.enter_context(tc.tile_pool(name="avst", bufs=2))
        ytr_pool = ctx.enter_context(tc.tile_pool(name="ytr", bufs=2))
        ost_pool = ctx.enter_context(tc.tile_pool(name="ost", bufs=6))

        # PSUM: psS 2x[128,1024] (4 banks) + av 2x[128,512] (2) + ps 2x[128,512] (2)
        ps = ctx.enter_context(tc.tile_pool(name="ps", bufs=2, space="PSUM"))
        av_pool = ctx.enter_context(tc.tile_pool(name="av", bufs=2, space="PSUM"))
        psS_pool = ctx.enter_context(tc.tile_pool(name="psS", bufs=2, space="PSUM"))

        v65_4d = v65_sb[:].rearrange("p (t h d) -> p t h d", t=NTT, h=HPC, d=65)

        def xA(c, t4):  # xT chunk for stage A: [128, 512] (c-tile, query chunk)
            return xT_sb[:, t4 * TCH + c * 512: t4 * TCH + (c + 1) * 512]

        def xB(c, t):   # xT tile for v: [128, 128] (c-tile, key tile)
            t4, r = divmod(t, 4)
            return xT_sb[:, t4 * TCH + c * 512 + r * 128: t4 * TCH + c * 512 + (r + 1) * 128]

        def w1(c, b):   # q0/k0 band chunk (b in 0,1) or v chunk (b == 2)
            w = 128 if b < 2 else 256
            return wqkv1_sb[:, c * 512 + b * 128: c * 512 + b * 128 + w]

        def w2(c, b):   # q1/k1 band chunk (b in 0,1)
            return wqkv2_sb[:, c * 256 + b * 128: c * 256 + (b + 1) * 128]

        # ---- input DMA stream (sync=HWDGE ring, FIFO). Everything
        # partition-major: long contiguous per-partition runs = line rate.
        nc.gpsimd.memset(scr_sb[:], 0.0)
        nc.gpsimd.dma_start(
            v65_4d[:, :, :, 64:65],
            vones.ap().rearrange("p (t h o) -> p t h o", t=NTT, h=HPC, o=1))
        nc.sync.dma_start(wqkv1_sb[:], wqkv1p[:])
        nc.sync.dma_start(xT_sb[:, 0:TCH], xTp[:, 0:TCH])
        nc.sync.dma_start(maskc_sb[:], maskc[:])
        nc.sync.dma_start(sel_sb[:], sel[:])
        nc.sync.dma_start(wqkv2_sb[:], wqkv2p[:])
        for t4 in range(1, 4):
            nc.sync.dma_start(xT_sb[:, t4 * TCH:(t4 + 1) * TCH],
                              xTp[:, t4 * TCH:(t4 + 1) * TCH])
        nc.sync.dma_start(wp_sb[:], wpp[:])

        # warm-up: keep the PE busy during the input-DMA wait so the HAM
        # clock gate reaches 8/8 before (and stays through) the real work.
        for i in range(24):
            warm = psS_pool.tile([128, 1024], F32, tag="psS", name=f"warm_{i}")
            nc.tensor.matmul(warm[:, 0:512], scr_sb[:, 0:128], scr_sb[:],
                             start=True, stop=True)

        # ---- band task: one qk band (128 cols) x one query chunk (512) ----
        def emit_band(b, t4):
            acc = ps.tile([128, 512], F32, tag="ps", name=f"accA_{b}_{t4}")
            for c in range(NCT):
                lhs = w1(c, b) if b < 2 else w2(c, b - 2)
                nc.tensor.matmul(acc[:], lhs, xA(c, t4),
                                 start=(c == 0), stop=(c == NCT - 1))
            nc.vector.tensor_copy(qkT_sb[:, b * T + t4 * 512: b * T + (t4 + 1) * 512], acc[:])

        # ---- v task: v natural [t, j] for one k-tile (xT stationary) ----
        def emit_B(t):
            psv = ps.tile([128, 512], F32, tag="ps", name=f"psv_{t}")
            for c in range(NCT):
                nc.tensor.matmul(psv[:, 0:256], xB(c, t), w1(c, 2),
                                 start=(c == 0), stop=(c == NCT - 1))
            dst = v65_4d[:, t, :, 0:64]
            src_ = psv[:, 0:256].rearrange("p (h d) -> p h d", h=HPC, d=64)
            nc.vector.tensor_copy(dst, src_)

        # ---- projection group: out^T[n-chunk, q-chunk] with wp stationary --
        nproj = [0]
        tail_mode = [False]

        def emit_proj(qc, n):
            pso = ps.tile([128, 512], F32, tag="ps", name=f"pso_{qc}_{n}")
            for p in range(NPAIR):
                lhsT = wp_sb[:, p * C + n * 128: p * C + (n + 1) * 128]
                rhs = yt_sb[:, p * T + qc * 512: p * T + (qc + 1) * 512]
                nc.tensor.matmul(pso[:], lhsT, rhs, start=(p == 0), stop=(p == NPAIR - 1))
            ost = ost_pool.tile([128, 512], BF16, tag="ost", name=f"ost_{qc}_{n}")
            # in the tail the exp stream is done, so alternate the PSUM->SBUF
            # copy between DVE and the now-idle ACT engine so back-to-back
            # projection groups don't serialize on one copy engine.
            if tail_mode[0] and nproj[0] % 2 == 1:
                nc.scalar.copy(ost[:], pso[:])
            else:
                nc.vector.tensor_copy(ost[:], pso[:])
            eng = nc.sync if (tail_mode[0] and nproj[0] % 2 == 0) else nc.gpsimd
            nproj[0] += 1
            eng.dma_start(outT[n * 128:(n + 1) * 128, qc * 512:(qc + 1) * 512], ost[:])

        # ---- per-pair normalization, split into DVE part and PE part ----
        # everything lives on partitions 0:2 (the eviction DMA shifts the
        # sums rows there), so DVE ops stay partition-aligned and the sel
        # broadcast matrix is one shared [2,128] block.
        def norm_dve(qc, p, sums2, rcs):
            rc2f = rc_pool.tile([2, 512], F32, tag="rcf", name=f"rcf_{qc}_{p}")
            rc2b = rc_pool.tile([2, 512], BF16, tag="rcb", name=f"rcb_{qc}_{p}")
            nc.vector.reciprocal_approx_fast(rc2f[:], sums2[:])
            nc.vector.tensor_copy(rc2b[:], rc2f[:])
            rcs.append(rc2b)

        def norm_pe(qc, p, rcs, ytr):
            psR = ps.tile([128, 512], F32, tag="ps", name=f"psR_{p}_{qc}")
            nc.tensor.matmul(psR[:], sel_sb[0:2, 0:128], rcs[0][:],
                             start=True, stop=True)
            nc.vector.tensor_mul(yt_sb[:, p * T + qc * 512: p * T + (qc + 1) * 512],
                                 ytr[:], psR[:])

        # ---- filler scheduler: tasks sorted by emission deadline (seg, kt)
        # where seg = 2*qc+p of the attention segment and kt the iteration
        # within it before which the task MUST have been emitted (tile deps
        # only exist for already-emitted writers).
        tasks = []  # sorted list of (deadline, cost_ns, seq, fn)
        seq = [0]
        debt = [0.0]

        def add_task(dl, cost, fn):
            bisect.insort(tasks, (dl, cost, seq[0], fn))
            seq[0] += 1

        def run_head():
            dl, cost, _, fn = tasks.pop(0)
            fn()
            debt[0] -= cost

        def drip(credit):
            debt[0] += credit
            while tasks and debt[0] > 0:
                run_head()
            # one oversized task must not starve later drip slots
            debt[0] = max(debt[0], -1200.0)

        def force(dl):
            while tasks and tasks[0][0] <= dl:
                run_head()

        # prologue: q0/k0 bands for qc0 + first two v tiles
        emit_band(0, 0)
        emit_band(1, 0)
        emit_B(0)
        emit_B(1)
        add_task((0, 2), 1750, lambda: emit_B(2))
        add_task((0, 3), 1750, lambda: emit_B(3))
        for t4 in range(1, 4):
            for t in range(4 * t4, 4 * t4 + 4):
                add_task((2 * t4, t), 1750, lambda t=t: emit_B(t))
        for t4 in range(4):
            for p in range(NPAIR):
                if (t4, p) == (0, 0):
                    continue
                # q-band (2p) chunk t4 needed at segment (qc=t4, p) start
                add_task((2 * t4 + p, -1), 1900, lambda p=p, t4=t4: emit_band(2 * p, t4))
                # k-band (2p+1) chunk t4 needed when emit_S(4*t4) is emitted,
                # i.e. during iteration 4*t4-2 of segment (qc=t4, p)
                dlk = (p, -1) if t4 == 0 else (2 * t4 + p, 4 * t4 - 2)
                add_task(dlk, 1900, lambda p=p, t4=t4: emit_band(2 * p + 1, t4))

        # ---- attention: qc-outer, pair-inner; S^T -> exp -> A@V ----
        for qc in range(NQC):
            nkt = 4 * qc + 4
            for p in range(NPAIR):
                seg = 2 * qc + p
                force((seg, -1))
                qb, kb = 2 * p, 2 * p + 1
                av = [av_pool.tile([128, 512], F32, tag="av", name=f"av_{p}_{qc}_{i}")
                      for i in range(2)]

                def emit_S(kt, qb=qb, kb=kb, qc=qc, p=p):
                    psb = psS_pool.tile([128, 1024], F32, tag="psS", name=f"psS_{p}_{qc}_{kt}")
                    slo = max(kt - 4 * qc, 0) * 128
                    for h in range(2):
                        base = 64 * h
                        lhsT = qkT_sb[base:base + 64, kb * T + kt * 128: kb * T + (kt + 1) * 128]
                        rhs = qkT_sb[base:base + 64, qb * T + qc * 512 + slo: qb * T + (qc + 1) * 512]
                        nc.tensor.matmul(psb[:, h * 512 + slo:(h + 1) * 512], lhsT, rhs,
                                         start=True, stop=True, tile_position=(base, 0))
                    return psb

                pipe = [emit_S(0)]
                if nkt > 1:
                    pipe.append(emit_S(1))
                for kt in range(nkt):
                    force((seg, kt))
                    cur = pipe.pop(0)
                    if kt + 2 < nkt:
                        pipe.append(emit_S(kt + 2))
                    d = kt - 4 * qc
                    lo = max(d, 0) * 128
                    psb2 = cur[:].rearrange("p (h q) -> p h q", h=2, q=512)
                    if d >= 0:
                        nc.vector.tensor_add(psb2[:, :, lo:lo + 128], psb2[:, :, lo:lo + 128],
                                             maskc_sb[:].rearrange("p (h q) -> p h q", h=2, q=128))
                    es = es_pool.tile([128, 1024], BF16, tag="es", name=f"es_{p}_{qc}_{kt}")
                    es2 = es[:].rearrange("p (h q) -> p h q", h=2, q=512)
                    nc.scalar.activation(es2[:, :, lo:], psb2[:, :, lo:], EXP, scale=SCALE)
                    for h in range(2):
                        hh = 2 * p + h
                        lhsT_v = v65_sb[:, kt * 260 + hh * 65: kt * 260 + (hh + 1) * 65]
                        nc.tensor.matmul(av[h][0:65, lo:], lhsT_v, es[:, h * 512 + lo:(h + 1) * 512],
                                         start=(kt == 0), stop=(kt == nkt - 1))
                    drip(650.0 * (512 - lo) / 512)
                # evict Y^T + sums (PSUM can't feed DMA: stage via SBUF; the
                # ytr partition shift rides the sync ring, the sums rows ride
                # the gpsimd ring so the two don't serialize).
                for i in range(2):
                    bw = psS_pool.tile([128, 1024], F32, tag="psS", name=f"bwarm_{p}_{qc}_{i}")
                    nc.tensor.matmul(bw[:, 0:512], scr_sb[:, 0:128], scr_sb[:],
                                     start=True, stop=True)
                ytr = ytr_pool.tile([128, 512], F32, tag="ytr", name=f"ytr_{p}_{qc}")
                sums2 = sums_pool.tile([2, 512], F32, tag="sums", name=f"sums_{qc}_{p}")
                sts = []
                for h in range(2):
                    st = avst_pool.tile([65, 512], F32, tag="avst", name=f"avst_{p}_{qc}_{h}")
                    # h1's copy rides the ACT engine (idle at segment ends) so
                    # the two eviction copies run in parallel
                    if h == 0:
                        nc.vector.tensor_copy(st[:], av[h][0:65, :])
                    else:
                        nc.scalar.copy(st[:], av[h][0:65, :])
                    sts.append(st)
                # sums rows first on the sync ring: they gate the reciprocal
                # chain; the ytr shifts' latency is hidden by the schedule
                for h in range(2):
                    nc.sync.dma_start(sums2[h:h + 1, :], sts[h][64:65, :])
                for h in range(2):
                    nc.sync.dma_start(ytr[64 * h:64 * (h + 1), :], sts[h][0:64, :])
                # pair p's normalization: DVE part one iteration into the
                # next segment (so the DVE queue head never blocks on the
                # sums-DMA latency), PE part a few iterations later. Pair 0's
                # whole chain runs during pair 1's attention; only pair 1's
                # lands after the last exp.
                rcs = []
                add_task((seg + 1, 1), 400,
                         lambda qc=qc, p=p, sums2=sums2, rcs=rcs: norm_dve(qc, p, sums2, rcs))
                add_task((seg + 1, 4), 900,
                         lambda qc=qc, p=p, rcs=rcs, ytr=ytr: norm_pe(qc, p, rcs, ytr))
            for n in range(NCT):
                add_task((2 * qc + 3, n), 550, lambda qc=qc, n=n: emit_proj(qc, n))
        for i in range(12):
            warm = psS_pool.tile([128, 1024], F32, tag="psS", name=f"tailwarm_{i}")
            nc.tensor.matmul(warm[:, 0:512], scr_sb[:, 0:128], scr_sb[:],
                             start=True, stop=True)
        tail_mode[0] = True
        force((1000, 0))


_NC_CACHE = None


def _get_nc():
    global _NC_CACHE
    if _NC_CACHE is None:
        _NC_CACHE = build_kernel()
    return _NC_CACHE


def _make_in_maps(x, w_attn, w_proj):
    import ml_dtypes
    bf16 = ml_dtypes.bfloat16
    x = np.asarray(x, dtype=np.float32)
    w_attn = np.asarray(w_attn, dtype=np.float32)
    w_proj = np.asarray(w_proj, dtype=np.float32)
    # maskc: strictly-lower-triangular NEG (row j = key, col i = query;
    # masked iff j > i), replicated for the 2 heads of a pair. Added before
    # the exp's scale is applied, so pre-divide by SCALE.
    tri = np.tril(np.full((128, 128), NEG, dtype=np.float32), -1) / SCALE
    maskc = np.concatenate([tri, tri], axis=1)
    sel = np.zeros((2, 128), dtype=np.float32)
    for m in range(128):
        sel[m // 64, m] = 1.0
    vones = np.ones((128, 64), dtype=bf16)
    sel = sel.astype(bf16)
    in_maps = []
    for core in range(NCORES):
        b, g = core // 4, core % 4
        hs = g * HPC
        q_cols = w_attn[:, hs * HD:(hs + HPC) * HD]
        k_cols = w_attn[:, C + hs * HD: C + (hs + HPC) * HD]
        v_cols = w_attn[:, 2 * C + hs * HD: 2 * C + (hs + HPC) * HD]
        # partition-major pre-arrangements (row p = SBUF partition p):
        # wqkv1[p, (c, q0|k0|v)] ; wqkv2[p, (c, q1|k1)] ; xTp[p, (t4, c, 512)]
        wqkv1 = np.concatenate(
            [q_cols[:, 0:128], k_cols[:, 0:128], v_cols], axis=1).astype(bf16)
        wqkv2 = np.concatenate(
            [q_cols[:, 128:256], k_cols[:, 128:256]], axis=1).astype(bf16)
        wqkv1p = wqkv1.reshape(NCT, 128, 512).transpose(1, 0, 2).reshape(128, NCT * 512)
        wqkv2p = wqkv2.reshape(NCT, 128, 256).transpose(1, 0, 2).reshape(128, NCT * 256)
        xT = np.ascontiguousarray(x[b].T).astype(bf16)          # [1024, 2048]
        xTp = xT.reshape(NCT, 128, NQC, 512).transpose(1, 2, 0, 3).reshape(128, NQC * NCT * 512)
        wpc = w_proj[hs * HD:(hs + HPC) * HD, :].astype(bf16)   # [256, 1024]
        wpp = wpc.reshape(2, 128, C).transpose(1, 0, 2).reshape(128, 2 * C)
        in_maps.append({
            "xTp": np.ascontiguousarray(xTp),
            "wqkv1p": np.ascontiguousarray(wqkv1p),
            "wqkv2p": np.ascontiguousarray(wqkv2p),
            "wpp": np.ascontiguousarray(wpp),
            "maskc": maskc,
            "sel": sel,
            "vones": vones,
        })
    return in_maps


def run_cores(x, w_attn, w_proj, trace=False):
    nc = _get_nc()
    in_maps = _make_in_maps(x, w_attn, w_proj)
    res = run_bass_kernel_spmd(nc, in_maps, core_ids=list(range(NCORES)), trace=trace)
    out = np.zeros((B, T, C), dtype=np.float32)
    for core in range(NCORES):
        out[core // 4] += np.asarray(res.results[core]["outT"], dtype=np.float32).T
    return out, res


def kernel(x, w_attn, w_proj):
    out, _ = run_cores(x, w_attn, w_proj, trace=False)
    return out
